# revision 1
# baseline (speedup 1.0000x reference)
"""Two-layer GAT on Trainium2, sharded over 8 NeuronCores.

Strategy (per spec sharding_hint, adapted):
  - Nodes are split into 8 equal contiguous shards (6250 each); edges are
    sorted by dst and each core owns the edges whose dst falls in its shard.
  - Layer-1 dense projection (x @ W1aug, W1aug = [W1 | W1@al_bd | W1@ar_bd])
    is replicated on every core (cheap on PE; avoids a 53MB all-gather).
    This produces the full gather table Haug1[n] = [h(256) | el(4) | er(4)].
  - Edge phase: for each block of 128 consecutive dst nodes, gather the
    src rows of Haug1 with large multi-row indirect DMAs, compute
    w = exp(leaky_relu(el[src]+er[dst])) on-chip, and segment-sum with
    selection-matrix matmuls accumulating into PSUM (psum row = dst slot).
    Softmax normalization happens once per node after accumulation
    (out = agg/denom); exp() without max-subtraction is numerically safe
    for this data (|e| < ~40 << 88).
  - Layer-2 dense is fused into the layer-1 edge phase per block; the small
    layer-2 table [50000, 66] is AllGathered across cores; the layer-2 edge
    phase then writes the final per-shard output.
  - All per-core variability (src/dst indices, segment ids, padding) is in
    uploaded metadata arrays so one SPMD program serves all 8 cores.
"""

import math

import numpy as np

import concourse.bass as bass
import concourse.bacc as bacc
import concourse.mybir as mybir
import concourse.tile as tile
from concourse.bass import IndirectOffsetOnAxis
from concourse.bass_utils import run_bass_kernel_spmd
from concourse.masks import make_identity

F32 = mybir.dt.float32
I32 = mybir.dt.int32
AF = mybir.ActivationFunctionType
OP = mybir.AluOpType

P = 128
NCORES = 8

# problem constants (hardcoded per spec)
N = 50000
E = 800000
IN = 256
HID = 64
OUT = 64
H1 = 4
H2 = 1
SLOPE = 0.2


def _ceil_div(a, b):
    return (a + b - 1) // b


def preprocess_edges(src, dst, n_nodes, shard, nb):
    """Sort edges by dst, shard by dst range, pack into uniform
    (core, block, tile) grid. Returns per-core metadata arrays and TPB.

    For core c, block b (128 consecutive dst nodes), the edges are laid out
    in column range [b*TPB, (b+1)*TPB) of [128, NB*TPB] arrays:
      srcoff: global src node id (gather row), 0 for padding
      dstoff: global dst node id (er gather row), 0 for padding
      segid:  dst slot within block (0..127), 999 for padding
    """
    src = np.asarray(src).astype(np.int64)
    dst = np.asarray(dst).astype(np.int64)
    order = np.argsort(dst, kind="stable")
    ssrc = src[order]
    sdst = dst[order]

    core_of = sdst // shard
    block_of = (sdst % shard) // P

    # counts per (core, block)
    flat = core_of * nb + block_of
    counts = np.bincount(flat, minlength=NCORES * nb).reshape(NCORES, nb)
    tpb = max(1, int(_ceil_div(counts.max(), P)))

    srcoffs, dstoffs, segids = [], [], []
    # edges are dst-sorted so (core, block) groups are contiguous
    starts = np.zeros(NCORES * nb + 1, dtype=np.int64)
    np.cumsum(counts.reshape(-1), out=starts[1:])
    for c in range(NCORES):
        so = np.zeros((P, nb * tpb), dtype=np.int32)
        do = np.zeros((P, nb * tpb), dtype=np.int32)
        sg = np.full((P, nb * tpb), 999.0, dtype=np.float32)
        for b in range(nb):
            lo = starts[c * nb + b]
            hi = starts[c * nb + b + 1]
            cnt = hi - lo
            if cnt == 0:
                continue
            s = np.arange(cnt)
            rows = s % P
            cols = b * tpb + s // P
            so[rows, cols] = ssrc[lo:hi]
            do[rows, cols] = sdst[lo:hi]
            sg[rows, cols] = (sdst[lo:hi] % shard) % P
        srcoffs.append(so)
        dstoffs.append(do)
        segids.append(sg)
    return srcoffs, dstoffs, segids, tpb


def build_program(nn, shard, inf, f1, f2, h1n, h2n, hid, out_d, nb, tpb,
                  skip_b1, slope):
    """Trace the SPMD Bass program. Returns nc."""
    a1 = f1 + 2 * h1n  # augmented width layer 1 (h | el | er)
    a2 = f2 + 2 * h2n  # layer 2
    ki1 = inf // P     # K chunks for layer-1 matmul
    ki2 = f1 // P      # K chunks for layer-2 matmul
    assert inf % P == 0 and f1 % P == 0

    nc = bacc.Bacc("TRN2", target_bir_lowering=False, debug=False,
                   num_devices=NCORES)

    xT = nc.dram_tensor("xT", [inf, nn], F32, kind="ExternalInput").ap()
    w1a = nc.dram_tensor("w1a", [inf, a1], F32, kind="ExternalInput").ap()
    w2a = nc.dram_tensor("w2a", [f1, a2], F32, kind="ExternalInput").ap()
    b1e = nc.dram_tensor("b1e", [P, f1], F32, kind="ExternalInput").ap()
    # bias folded for the elu(-1) shift plus b2: [P, a2] replicated
    bc2 = nc.dram_tensor("bc2", [P, a2], F32, kind="ExternalInput").ap()
    b2e = nc.dram_tensor("b2e", [P, f2], F32, kind="ExternalInput").ap()
    iota = nc.dram_tensor("iota", [P, P], F32, kind="ExternalInput").ap()
    soff = nc.dram_tensor("soff", [P, nb * tpb], I32, kind="ExternalInput").ap()
    doff = nc.dram_tensor("doff", [P, nb * tpb], I32, kind="ExternalInput").ap()
    segi = nc.dram_tensor("segi", [P, nb * tpb], F32, kind="ExternalInput").ap()
    outd = nc.dram_tensor("out", [shard, out_d], F32, kind="ExternalOutput").ap()

    haug1 = nc.dram_tensor("haug1", [nn, a1], F32, kind="Internal").ap()
    h2loc = nc.dram_tensor("h2loc", [shard, a2], F32, kind="Internal").ap()
    h2full = nc.dram_tensor("h2full", [nn, a2], F32, kind="Internal",
                            addr_space="Shared").ap()

    nt1 = _ceil_div(nn, P)

    with tile.TileContext(nc) as tc:
        with (
            tc.tile_pool(name="const", bufs=1) as cp,
            tc.tile_pool(name="xload", bufs=3) as xp,
            tc.tile_pool(name="stage", bufs=3) as sp,
            tc.tile_pool(name="gath", bufs=2) as gp,
            tc.tile_pool(name="small", bufs=3) as mp,
            tc.tile_pool(name="sel", bufs=4) as selp,
            tc.tile_pool(name="hwork", bufs=2) as hp,
            tc.tile_pool(name="psA", bufs=2, space="PSUM") as ppa,
            tc.tile_pool(name="psB", bufs=2, space="PSUM") as ppb,
            tc.tile_pool(name="psC", bufs=2, space="PSUM") as ppc,
        ):
            # ---- persistent constants ----
            w1sb = cp.tile([P, ki1, a1], F32)
            nc.sync.dma_start(out=w1sb[:], in_=w1a.rearrange("(k p) n -> p k n", p=P))
            w2sb = cp.tile([P, ki2, a2], F32)
            nc.sync.dma_start(out=w2sb[:], in_=w2a.rearrange("(k p) n -> p k n", p=P))
            b1sb = cp.tile([P, f1], F32)
            nc.sync.dma_start(out=b1sb[:], in_=b1e)
            bc2sb = cp.tile([P, a2], F32)
            nc.sync.dma_start(out=bc2sb[:], in_=bc2)
            b2sb = cp.tile([P, f2], F32)
            nc.sync.dma_start(out=b2sb[:], in_=b2e)
            iosb = cp.tile([P, P], F32)
            nc.sync.dma_start(out=iosb[:], in_=iota)
            idn = cp.tile([P, P], F32)
            make_identity(nc, idn[:])
            sosb = cp.tile([P, nb * tpb], I32)
            nc.sync.dma_start(out=sosb[:], in_=soff)
            dosb = cp.tile([P, nb * tpb], I32)
            nc.sync.dma_start(out=dosb[:], in_=doff)
            sgsb = cp.tile([P, nb * tpb], F32)
            nc.sync.dma_start(out=sgsb[:], in_=segi)

            # ---- phase D1: replicated dense layer 1 -> haug1 [nn, a1] ----
            for nt in range(nt1):
                m = min(P, nn - nt * P)
                xt = xp.tile([P, ki1, P], F32, tag="xt")
                nc.sync.dma_start(
                    out=xt[:, :, :m],
                    in_=xT[:, nt * P:nt * P + m].rearrange("(k p) n -> p k n", p=P),
                )
                ph = ppa.tile([P, a1], F32, space="PSUM", tag="acc")
                for k in range(ki1):
                    nc.tensor.matmul(out=ph[:m, :], lhsT=xt[:, k, :m],
                                     rhs=w1sb[:, k, :],
                                     start=(k == 0), stop=(k == ki1 - 1))
                st = sp.tile([P, a1], F32, tag="st")
                nc.scalar.copy(out=st[:m, :], in_=ph[:m, :])
                nc.sync.dma_start(out=haug1[nt * P:nt * P + m, :], in_=st[:m, :])

            # ---- phase E1 (+ fused dense layer 2) per 128-node block ----
            for b in range(nb):
                m = min(P, shard - b * P)
                cols = slice(b * tpb, (b + 1) * tpb)

                g = gp.tile([P, tpb, a1], F32, tag="g1")
                ere = mp.tile([P, tpb, h1n], F32, tag="ere")
                for t in range(tpb):
                    c = b * tpb + t
                    nc.gpsimd.indirect_dma_start(
                        out=g[:, t, :], out_offset=None, in_=haug1,
                        in_offset=IndirectOffsetOnAxis(ap=sosb[:, c:c + 1], axis=0),
                    )
                    nc.gpsimd.indirect_dma_start(
                        out=ere[:, t, :], out_offset=None, in_=haug1,
                        in_offset=IndirectOffsetOnAxis(ap=dosb[:, c:c + 1], axis=0),
                        element_offset=f1 + h1n,
                    )
                # e = el[src] + er[dst]; leaky_relu; w = exp(e)
                nc.vector.tensor_tensor(out=ere[:], in0=g[:, :, f1:f1 + h1n],
                                        in1=ere[:], op=OP.add)
                tmp = mp.tile([P, tpb, h1n], F32, tag="tmp")
                nc.vector.tensor_scalar_mul(out=tmp[:], in0=ere[:], scalar1=slope)
                nc.vector.tensor_tensor(out=ere[:], in0=ere[:], in1=tmp[:],
                                        op=OP.max)
                nc.scalar.activation(out=g[:, :, f1:f1 + h1n], in_=ere[:],
                                     func=AF.Exp)
                # X <- w * h[src] (per head)
                g4 = g[:, :, 0:f1].rearrange("p t (h d) -> p t h d", h=h1n)
                wb = g[:, :, f1:f1 + h1n].to_broadcast([P, tpb, h1n, hid])
                nc.vector.tensor_tensor(out=g4, in0=g4, in1=wb, op=OP.mult)

                # segment-sum via selection matmuls accumulating in PSUM
                pb = ppa.tile([P, f1 + h1n], F32, space="PSUM", tag="acc")
                for t in range(tpb):
                    sel = selp.tile([P, P], F32, tag="sel")
                    nc.vector.tensor_tensor(
                        out=sel[:],
                        in0=sgsb[:, b * tpb + t:b * tpb + t + 1].to_broadcast([P, P]),
                        in1=iosb[:], op=OP.is_equal)
                    nc.tensor.matmul(out=pb[:], lhsT=sel[:],
                                     rhs=g[:, t, 0:f1 + h1n],
                                     start=(t == 0), stop=(t == tpb - 1))

                # normalize: out = agg / denom (+eps to keep empty nodes at 0)
                den = mp.tile([P, h1n], F32, tag="den")
                nc.vector.tensor_scalar_add(out=den[:m], in0=pb[:m, f1:f1 + h1n],
                                            scalar1=1e-30)
                nc.vector.reciprocal(out=den[:m], in_=den[:m])
                h1t = hp.tile([P, f1], F32, tag="h1t")
                nc.vector.tensor_tensor(
                    out=h1t[:m].rearrange("p (h d) -> p h d", h=h1n),
                    in0=pb[:m, 0:f1].rearrange("p (h d) -> p h d", h=h1n),
                    in1=den[:m].to_broadcast([m, h1n, hid]), op=OP.mult)
                if not skip_b1:
                    nc.vector.tensor_tensor(out=h1t[:m], in0=h1t[:m],
                                            in1=b1sb[:m], op=OP.add)
                # elu (the -1 is folded into bc2): h = max(h,0) + exp(min(h,0))
                te = hp.tile([P, f1], F32, tag="te")
                nc.vector.tensor_scalar_min(out=te[:m], in0=h1t[:m], scalar1=0.0)
                nc.scalar.activation(out=te[:m], in_=te[:m], func=AF.Exp)
                nc.vector.tensor_scalar_max(out=h1t[:m], in0=h1t[:m], scalar1=0.0)
                nc.vector.tensor_tensor(out=h1t[:m], in0=h1t[:m], in1=te[:m],
                                        op=OP.add)

                # transpose h1 and project: h2aug = h1 @ w2aug + bc2
                h1T = hp.tile([P, ki2, P], F32, tag="h1T")
                for k in range(ki2):
                    pt = ppb.tile([P, P], F32, space="PSUM", tag="pt")
                    nc.tensor.transpose(out=pt[:, :m],
                                        in_=h1t[:m, k * P:(k + 1) * P],
                                        identity=idn[:m, :m])
                    nc.scalar.copy(out=h1T[:, k, :m], in_=pt[:, :m])
                p2 = ppc.tile([P, a2], F32, space="PSUM", tag="p2")
                for k in range(ki2):
                    nc.tensor.matmul(out=p2[:m, :], lhsT=h1T[:, k, :m],
                                     rhs=w2sb[:, k, :],
                                     start=(k == 0), stop=(k == ki2 - 1))
                st2 = sp.tile([P, a2], F32, tag="st2")
                nc.vector.tensor_tensor(out=st2[:m, :], in0=p2[:m, :],
                                        in1=bc2sb[:m, :], op=OP.add)
                nc.sync.dma_start(out=h2loc[b * P:b * P + m, :], in_=st2[:m, :])

            # ---- phase C2: all-gather layer-2 table ----
            nc.gpsimd.collective_compute(
                "AllGather", OP.bypass,
                replica_groups=[list(range(NCORES))],
                ins=[h2loc], outs=[h2full])

            # ---- phase E2: layer-2 edge phase -> final output ----
            for b in range(nb):
                m = min(P, shard - b * P)
                cols = slice(b * tpb, (b + 1) * tpb)

                g2 = gp.tile([P, tpb, a2], F32, tag="g2")
                er2 = mp.tile([P, tpb, h2n], F32, tag="er2")
                for t in range(tpb):
                    c = b * tpb + t
                    nc.gpsimd.indirect_dma_start(
                        out=g2[:, t, :], out_offset=None, in_=h2full,
                        in_offset=IndirectOffsetOnAxis(ap=sosb[:, c:c + 1], axis=0),
                    )
                    nc.gpsimd.indirect_dma_start(
                        out=er2[:, t, :], out_offset=None, in_=h2full,
                        in_offset=IndirectOffsetOnAxis(ap=dosb[:, c:c + 1], axis=0),
                        element_offset=f2 + h2n,
                    )
                w2t = mp.tile([P, tpb, h2n], F32, tag="w2t")
                nc.vector.tensor_tensor(out=w2t[:], in0=g2[:, :, f2:f2 + h2n],
                                        in1=er2[:], op=OP.add)
                tmp2 = mp.tile([P, tpb, h2n], F32, tag="tmp2")
                nc.vector.tensor_scalar_mul(out=tmp2[:], in0=w2t[:], scalar1=slope)
                nc.vector.tensor_tensor(out=w2t[:], in0=w2t[:], in1=tmp2[:],
                                        op=OP.max)
                nc.scalar.activation(out=w2t[:], in_=w2t[:], func=AF.Exp)
                # w folds into the selection matrix; denominator column <- 1.0
                nc.scalar.activation(out=g2[:, :, f2:f2 + h2n],
                                     in_=g2[:, :, f2:f2 + h2n],
                                     func=AF.Identity, bias=1.0, scale=0.0)

                pb2 = ppa.tile([P, a2 - h2n], F32, space="PSUM", tag="acc")
                for t in range(tpb):
                    sel = selp.tile([P, P], F32, tag="sel")
                    nc.vector.tensor_scalar(
                        out=sel[:], in0=iosb[:],
                        scalar1=sgsb[:, b * tpb + t:b * tpb + t + 1],
                        scalar2=w2t[:, t, 0:1],
                        op0=OP.is_equal, op1=OP.mult)
                    nc.tensor.matmul(out=pb2[:], lhsT=sel[:],
                                     rhs=g2[:, t, 0:f2 + h2n],
                                     start=(t == 0), stop=(t == tpb - 1))

                den2 = mp.tile([P, h2n], F32, tag="den2")
                nc.vector.tensor_scalar_add(out=den2[:m], in0=pb2[:m, f2:f2 + h2n],
                                            scalar1=1e-30)
                nc.vector.reciprocal(out=den2[:m], in_=den2[:m])
                of = sp.tile([P, out_d], F32, tag="of")
                nc.vector.tensor_tensor(
                    out=of[:m].rearrange("p (h d) -> p h d", h=h2n),
                    in0=pb2[:m, 0:f2].rearrange("p (h d) -> p h d", h=h2n),
                    in1=den2[:m].to_broadcast([m, h2n, out_d]), op=OP.mult)
                nc.vector.tensor_tensor(out=of[:m], in0=of[:m], in1=b2sb[:m],
                                        op=OP.add)
                nc.sync.dma_start(out=outd[b * P:b * P + m, :], in_=of[:m, :])

    nc.compile()
    return nc


def make_inputs(features, src, dst, W1, al1, ar1, b1, W2, al2, ar2, b2,
                n_nodes, shard, inf, f1, f2, h1n, h2n, hid, out_d):
    """Host-side preprocessing: build per-core input dicts. Returns
    (in_maps, tpb, nb, skip_b1)."""
    nb = _ceil_div(shard, P)

    features = np.ascontiguousarray(np.asarray(features, dtype=np.float32))
    W1 = np.asarray(W1, dtype=np.float32)
    W2 = np.asarray(W2, dtype=np.float32)
    al1 = np.asarray(al1, dtype=np.float32).reshape(h1n, hid)
    ar1 = np.asarray(ar1, dtype=np.float32).reshape(h1n, hid)
    al2 = np.asarray(al2, dtype=np.float32).reshape(h2n, out_d)
    ar2 = np.asarray(ar2, dtype=np.float32).reshape(h2n, out_d)
    b1 = np.asarray(b1, dtype=np.float32).reshape(-1)
    b2 = np.asarray(b2, dtype=np.float32).reshape(-1)

    # block-diag head maps: al_bd [f1, h1n], al_bd[h*hid+d, h] = al1[h, d]
    def blockdiag(a, heads, d):
        m = np.zeros((heads * d, heads), dtype=np.float32)
        for h in range(heads):
            m[h * d:(h + 1) * d, h] = a[h]
        return m

    w1aug = np.concatenate(
        [W1, W1 @ blockdiag(al1, h1n, hid), W1 @ blockdiag(ar1, h1n, hid)],
        axis=1)  # [inf, a1]
    w2aug = np.concatenate(
        [W2, W2 @ blockdiag(al2, h2n, out_d), W2 @ blockdiag(ar2, h2n, out_d)],
        axis=1)  # [f1, a2]

    # elu's -1 shift folded through W2aug: (X-1)@W = X@W - colsum(W)
    bc2 = np.tile(-w2aug.sum(axis=0, keepdims=True), (P, 1)).astype(np.float32)
    b1e = np.tile(b1[None, :], (P, 1)).astype(np.float32)
    b2e = np.tile(b2[None, :], (P, 1)).astype(np.float32)
    iota = np.tile(np.arange(P, dtype=np.float32)[None, :], (P, 1))
    xT = np.ascontiguousarray(features.T)  # [inf, nn]

    skip_b1 = not np.any(b1)

    srcoffs, dstoffs, segids, tpb = preprocess_edges(src, dst, n_nodes, shard, nb)

    in_maps = []
    for c in range(NCORES):
        in_maps.append({
            "xT": xT,
            "w1a": np.ascontiguousarray(w1aug),
            "w2a": np.ascontiguousarray(w2aug),
            "b1e": b1e,
            "bc2": bc2,
            "b2e": b2e,
            "iota": iota,
            "soff": srcoffs[c],
            "doff": dstoffs[c],
            "segi": segids[c],
        })
    return in_maps, tpb, nb, skip_b1


def _run(features, src, dst, W1, al1, ar1, b1, W2, al2, ar2, b2, **spmd_kwargs):
    f1 = H1 * HID
    f2 = H2 * OUT
    shard = N // NCORES
    in_maps, tpb, nb, skip_b1 = make_inputs(
        features, src, dst, W1, al1, ar1, b1, W2, al2, ar2, b2,
        N, shard, IN, f1, f2, H1, H2, HID, OUT)
    nc = build_program(N, shard, IN, f1, f2, H1, H2, HID, OUT, nb, tpb,
                       skip_b1, SLOPE)
    res = run_bass_kernel_spmd(nc, in_maps, core_ids=list(range(NCORES)),
                               **spmd_kwargs)
    out = np.concatenate([res.results[c]["out"] for c in range(NCORES)], axis=0)
    return out.astype(np.float32), res


def kernel(features, src, dst, W1, al1, ar1, b1, W2, al2, ar2, b2):
    out, _ = _run(features, src, dst, W1, al1, ar1, b1, W2, al2, ar2, b2)
    return out


def run_timed(features, src, dst, W1, al1, ar1, b1, W2, al2, ar2, b2,
              iters=5):
    """Run like kernel(), but keep inputs device-resident and time repeated
    executions of the compiled NEFF. Returns (out, best_wall_ns)."""
    import time as _time

    import jax
    from jax.sharding import Mesh, PartitionSpec
    from jax.experimental.shard_map import shard_map
    from concourse.bass2jax import (_bass_exec_p, install_neuronx_cc_hook,
                                    partition_id_tensor)

    f1 = H1 * HID
    f2 = H2 * OUT
    shard = N // NCORES
    in_maps, tpb, nb, skip_b1 = make_inputs(
        features, src, dst, W1, al1, ar1, b1, W2, al2, ar2, b2,
        N, shard, IN, f1, f2, H1, H2, HID, OUT)
    nc = build_program(N, shard, IN, f1, f2, H1, H2, HID, OUT, nb, tpb,
                       skip_b1, SLOPE)

    install_neuronx_cc_hook()
    part_name = (nc.partition_id_tensor.name if nc.partition_id_tensor
                 else None)
    in_names, out_names, out_avals, zero_outs = [], [], [], []
    for alloc in nc.m.functions[0].allocations:
        if not isinstance(alloc, mybir.MemoryLocationSet):
            continue
        name = alloc.memorylocations[0].name
        if alloc.kind == "ExternalInput":
            if name != part_name:
                in_names.append(name)
        elif alloc.kind == "ExternalOutput":
            out_names.append(name)
            shp = tuple(alloc.tensor_shape)
            dt = mybir.dt.np(alloc.dtype)
            out_avals.append(jax.core.ShapedArray(shp, dt))
            zero_outs.append(np.zeros(shp, dt))
    n_params = len(in_names)
    all_names = in_names + out_names
    if part_name is not None:
        all_names = all_names + [part_name]

    def _body(*args):
        operands = list(args)
        if part_name is not None:
            operands.append(partition_id_tensor())
        return tuple(_bass_exec_p.bind(
            *operands, out_avals=tuple(out_avals), in_names=tuple(all_names),
            out_names=tuple(out_names), lowering_input_output_aliases=(),
            sim_require_finite=True, sim_require_nnan=True, nc=nc))

    devices = jax.devices()[:NCORES]
    mesh = Mesh(np.asarray(devices), ("core",))
    specs = (PartitionSpec("core"),) * (n_params + len(out_names))
    out_specs = (PartitionSpec("core"),) * len(out_names)
    fn = jax.jit(shard_map(_body, mesh=mesh, in_specs=specs,
                           out_specs=out_specs, check_rep=False),
                 keep_unused=True)

    concat_in = [np.concatenate([in_maps[c][nm] for c in range(NCORES)], axis=0)
                 for nm in in_names]
    concat_zero = [np.concatenate([z] * NCORES, axis=0) for z in zero_outs]
    args = [jax.device_put(a) for a in concat_in + concat_zero]
    outs = fn(*args)  # compile + warm up
    jax.block_until_ready(outs)

    def timed_chain(k):
        best = None
        for _ in range(iters):
            t0 = _time.perf_counter_ns()
            for _ in range(k):
                outs = fn(*args)
            jax.block_until_ready(outs)
            dt = _time.perf_counter_ns() - t0
            best = dt if best is None else min(best, dt)
        return best

    t1 = timed_chain(1)
    t6 = timed_chain(6)
    slope = max((t6 - t1) // 5, 1)
    print(f"[timing] 1-call wall: {t1/1e6:.2f} ms; 6-call wall: {t6/1e6:.2f} ms; "
          f"marginal per-exec: {slope/1e6:.3f} ms", flush=True)
    outs = fn(*args)
    jax.block_until_ready(outs)
    out_full = np.asarray(outs[out_names.index("out")])
    return out_full.astype(np.float32), slope



# revision 25
# speedup vs baseline: 1.7332x; 1.7332x over previous
"""Two-layer GAT on Trainium2, sharded over 8 NeuronCores.

Strategy:
  - Nodes split into 8 contiguous shards (6250/core); edges sorted by dst and
    owned by the core whose shard contains the dst.
  - The dense projections are replicated (cheap in bf16 on PE); each core
    writes the full gather table haug[n] = [h(256) | el(4) | er(4) | pad] in
    bf16 (row = 384 elems = 768 B, a multiple of the 256 B SWDGE-gather
    granule).
  - Edge phase: per block of 128 consecutive dst nodes, ALL of the block's
    src rows are fetched with two InstDMAGatherAnt ops (int16 indices limit
    one gather to 32768 table rows, so the table is addressed as a lo view
    [0:32768) and a hi view [32768:N)); er[dst] is fetched with ONE batched
    indirect DMA (int32 offsets, element_offset picks the er columns).
    w = exp(leaky_relu(el_src + er_dst)) on-chip; segment-sum via per-column
    selection-matrix matmuls (bf16) accumulating in PSUM; softmax normalizes
    once per node after accumulation (exp without max-subtraction is safe
    here: |e| << 88).
  - Layer-2 projection is fused per block; the small layer-2 table
    [N, 128]bf16 is AllGathered; the layer-2 edge phase mirrors layer 1.
  - All per-core variability lives in uploaded metadata (indices, segids),
    so one SPMD program serves all 8 cores.
"""

import numpy as np
import ml_dtypes

import concourse.bass as bass
import concourse.bacc as bacc
import concourse.mybir as mybir
import concourse.tile as tile
from concourse.bass import IndirectOffsetOnAxis
from concourse.bass_utils import run_bass_kernel_spmd

F32 = mybir.dt.float32
BF16 = mybir.dt.bfloat16
I16 = mybir.dt.int16
I32 = mybir.dt.int32
AF = mybir.ActivationFunctionType
OP = mybir.AluOpType

P = 128
NCORES = 8

# problem constants (hardcoded per spec)
N = 50000
E = 800000
IN = 256
HID = 64
OUT = 64
H1 = 4
H2 = 1
SLOPE = 0.2

F1 = H1 * HID          # 256
F2 = H2 * OUT          # 64
C1 = F1 + 2 * H1       # 264 real cols of layer-1 table row
C2 = F2 + 2 * H2       # 66 real cols of layer-2 table row
A1 = 384               # padded bf16 row width, layer-1 table (768 B)
A2 = 128               # padded bf16 row width, layer-2 table (256 B)
NLO = 32768            # int16 gather-index range split point
SHARD = N // NCORES    # 6250
NB = (SHARD + P - 1) // P  # 49


def _ceil_div(a, b):
    return (a + b - 1) // b


def _wrap_idx16(vals, ncols):
    """[n] values -> int16 [128, ncols] wrapped in 16 partitions (position i
    at [i%16, i//16]) and replicated x8 across the 128 partitions."""
    flat = np.zeros(16 * ncols, np.int16)
    flat[:len(vals)] = vals
    out = np.ascontiguousarray(flat.reshape(ncols, 16).T)
    return np.tile(out, (8, 1))


def preprocess_edges(src, dst):
    """Sort edges by dst, shard by dst range, split each (core, block)'s
    edges by src < NLO, pack into a uniform (core, block, column) grid.

    Returns per-core metadata arrays and (tlo, thi).
    """
    src = np.asarray(src).astype(np.int64)
    dst = np.asarray(dst).astype(np.int64)
    order = np.argsort(dst, kind="stable")
    ssrc = src[order]
    sdst = dst[order]

    core_of = sdst // SHARD
    block_of = (sdst % SHARD) // P
    is_lo = ssrc < NLO

    flat = (core_of * NB + block_of) * 2 + (~is_lo)
    counts = np.bincount(flat, minlength=NCORES * NB * 2)
    clo = counts[0::2].reshape(NCORES, NB)
    chi = counts[1::2].reshape(NCORES, NB)
    tlo = max(1, int(_ceil_div(clo.max(), P)))
    thi = max(1, int(_ceil_div(chi.max(), P)))
    t = tlo + thi

    # order edges by (core, block, hi/lo) so groups are contiguous
    order2 = np.lexsort((~is_lo, block_of, core_of))
    ssrc = ssrc[order2]
    sdst = sdst[order2]
    starts = np.zeros(NCORES * NB * 2 + 1, dtype=np.int64)
    np.cumsum(counts, out=starts[1:])

    idx16s, dlo16s, dhi16s, segids, mlos, mhis = [], [], [], [], [], []
    for c in range(NCORES):
        ix = np.zeros((128, NB * t * 8), np.int16)
        dl = np.zeros((128, NB * t * 8), np.int16)
        dh = np.zeros((128, NB * t * 8), np.int16)
        sg = np.full((P, NB * t), 1000.0, np.float32)
        ml = np.zeros((P, NB * t), np.float32)
        mh = np.zeros((P, NB * t), np.float32)
        for b in range(NB):
            base = (c * NB + b) * 2
            for half, (toff, tcols) in enumerate(((0, tlo), (tlo, thi))):
                lo = starts[base + half]
                hi = starts[base + half + 1]
                cnt = hi - lo
                assert cnt <= tcols * P
                sval = ssrc[lo:hi] - (NLO if half else 0)
                dval = sdst[lo:hi]
                c0 = (b * t + toff) * 8
                c1 = (b * t + toff + tcols) * 8
                ix[:, c0:c1] = _wrap_idx16(sval, tcols * 8)
                dl[:, c0:c1] = _wrap_idx16(np.minimum(dval, NLO - 1),
                                           tcols * 8)
                dh[:, c0:c1] = _wrap_idx16(np.maximum(dval - NLO, 0),
                                           tcols * 8)
                s = np.arange(cnt)
                rows = s % P
                cols = b * t + toff + s // P
                sg[rows, cols] = (dval % SHARD) % P
                ml[rows, cols] = (dval < NLO).astype(np.float32)
                mh[rows, cols] = (dval >= NLO).astype(np.float32)
        idx16s.append(ix)
        dlo16s.append(dl)
        dhi16s.append(dh)
        segids.append(sg)
        mlos.append(ml)
        mhis.append(mh)
    return idx16s, dlo16s, dhi16s, segids, mlos, mhis, tlo, thi


def build_program(tlo, thi, skip_b1, skip_b2, nocoll=False, parts="full"):
    """Trace the SPMD Bass program. Returns nc.

    parts: "full" | "d1" (dense only) | "e1" (dense + layer-1 edge) |
           "e1g" (e1 but only src gathers, er zeroed) |
           "e1e" (e1 but src gathers skipped, er fetched) |
           "fullN<k>" (full but only k edge blocks per phase)"""
    nb_run = NB
    if parts.startswith("fullN"):
        nb_run = int(parts[5:])
        parts = "full"
    t = tlo + thi
    ki1 = IN // P   # 2
    ki2 = F1 // P   # 2
    nt1 = _ceil_div(N, P)  # 391

    nc = bacc.Bacc("TRN2", target_bir_lowering=False, debug=False,
                   num_devices=NCORES, num_swdge_queues=3)

    xTb = nc.dram_tensor("xTb", [IN, N], BF16, kind="ExternalInput").ap()
    w1a = nc.dram_tensor("w1a", [IN, C1], BF16, kind="ExternalInput").ap()
    w2a = nc.dram_tensor("w2a", [F1, C2], BF16, kind="ExternalInput").ap()
    b1e = nc.dram_tensor("b1e", [P, F1], F32, kind="ExternalInput").ap()
    bc2 = nc.dram_tensor("bc2", [P, C2], F32, kind="ExternalInput").ap()
    b2e = nc.dram_tensor("b2e", [P, F2], F32, kind="ExternalInput").ap()
    iota = nc.dram_tensor("iota", [P, P], BF16, kind="ExternalInput").ap()
    idxs = nc.dram_tensor("idxs", [P, NB * t * 8], I16, kind="ExternalInput").ap()
    dlo = nc.dram_tensor("dlo", [P, NB * t * 8], I16, kind="ExternalInput").ap()
    dhi = nc.dram_tensor("dhi", [P, NB * t * 8], I16, kind="ExternalInput").ap()
    segi = nc.dram_tensor("segi", [P, NB * t], BF16, kind="ExternalInput").ap()
    mlo = nc.dram_tensor("mlo", [P, NB * t], BF16, kind="ExternalInput").ap()
    mhi = nc.dram_tensor("mhi", [P, NB * t], BF16, kind="ExternalInput").ap()
    outd = nc.dram_tensor("out", [SHARD, F2], F32, kind="ExternalOutput").ap()

    haug = nc.dram_tensor("haug", [N, A1], BF16, kind="Internal").ap()
    # compact [el(4) | er(4) | pad] rows: full-row 256B gathers for er[dst]
    # (sub-row elem_step gathers return garbage on HW)
    eler1 = nc.dram_tensor("eler1", [N, A2], BF16, kind="Internal").ap()
    h2loc = nc.dram_tensor("h2loc", [SHARD, A2], BF16, kind="Internal").ap()
    h2full = nc.dram_tensor("h2full", [N, A2], BF16, kind="Internal",
                            addr_space="Shared").ap()

    with tile.TileContext(nc) as tc:
        with (
            tc.tile_pool(name="const", bufs=1) as cp,
            tc.tile_pool(name="xload", bufs=3) as xp,
            tc.tile_pool(name="stage", bufs=3) as sp,
            tc.tile_pool(name="gath", bufs=2) as gp,
            tc.tile_pool(name="g2p", bufs=2) as g2p,
            tc.tile_pool(name="small", bufs=3) as mp,
            tc.tile_pool(name="sel", bufs=2) as selp,
            tc.tile_pool(name="hwork", bufs=2) as hp,
            tc.tile_pool(name="psA", bufs=2, space="PSUM") as ppa,
            tc.tile_pool(name="psB", bufs=2, space="PSUM") as ppb,
            tc.tile_pool(name="psC", bufs=2, space="PSUM") as ppc,
        ):
            # ---- persistent constants / metadata ----
            w1sb = cp.tile([P, ki1, C1], BF16)
            nc.sync.dma_start(out=w1sb[:], in_=w1a.rearrange("(k p) n -> p k n", p=P))
            w2sb = cp.tile([P, ki2, C2], BF16)
            nc.sync.dma_start(out=w2sb[:], in_=w2a.rearrange("(k p) n -> p k n", p=P))
            b1sb = cp.tile([P, F1], F32)
            nc.sync.dma_start(out=b1sb[:], in_=b1e)
            bc2sb = cp.tile([P, C2], F32)
            nc.sync.dma_start(out=bc2sb[:], in_=bc2)
            b2sb = cp.tile([P, F2], F32)
            nc.sync.dma_start(out=b2sb[:], in_=b2e)
            iosb = cp.tile([P, P], BF16)
            nc.sync.dma_start(out=iosb[:], in_=iota)
            from concourse.masks import make_identity
            idn = cp.tile([P, P], F32)
            make_identity(nc, idn[:])
            ixsb = cp.tile([P, NB * t * 8], I16)
            nc.sync.dma_start(out=ixsb[:], in_=idxs)
            dlsb = cp.tile([P, NB * t * 8], I16)
            nc.sync.dma_start(out=dlsb[:], in_=dlo)
            dhsb = cp.tile([P, NB * t * 8], I16)
            nc.sync.dma_start(out=dhsb[:], in_=dhi)
            sgsb = cp.tile([P, NB * t], BF16)
            nc.sync.dma_start(out=sgsb[:], in_=segi)
            mlsb = cp.tile([P, NB * t], BF16)
            nc.sync.dma_start(out=mlsb[:], in_=mlo)
            mhsb = cp.tile([P, NB * t], BF16)
            nc.sync.dma_start(out=mhsb[:], in_=mhi)

            qctr = [0]

            def gather_cols(out_tile, in_ap, idx_tile, blk, toff, ncols,
                            elem, elem_step=None):
                """Chunked dma_gather of `ncols` 128-row columns starting at
                column `toff` of block `blk` (ring cap: <=8 cols/instr)."""
                done = 0
                while done < ncols:
                    cc = min(8, ncols - done)
                    col = blk * t + toff + done
                    nc.gpsimd.dma_gather(
                        out_ap=out_tile[:, toff + done:toff + done + cc, :],
                        in_ap=in_ap,
                        idxs_ap=idx_tile[:, col * 8:(col + cc) * 8],
                        num_idxs=cc * P, num_idxs_reg=cc * P,
                        elem_size=elem, elem_step=elem_step,
                        queue_num=qctr[0] % 3)
                    qctr[0] += 1
                    done += cc

            # ---- phase D1: replicated dense layer 1 -> haug [N, A1] bf16 ----
            for nt in range(nt1):
                m = min(P, N - nt * P)
                xt = xp.tile([P, ki1, P], BF16, tag="xt")
                nc.sync.dma_start(
                    out=xt[:, :, :m],
                    in_=xTb[:, nt * P:nt * P + m].rearrange("(k p) n -> p k n", p=P),
                )
                ph = ppa.tile([P, C1], F32, space="PSUM", tag="acc")
                for k in range(ki1):
                    nc.tensor.matmul(out=ph[:m, :], lhsT=xt[:, k, :m],
                                     rhs=w1sb[:, k, :],
                                     start=(k == 0), stop=(k == ki1 - 1))
                st = sp.tile([P, C1], BF16, tag="st")
                nc.scalar.copy(out=st[:m, :], in_=ph[:m, :])
                nc.sync.dma_start(out=haug[nt * P:nt * P + m, 0:C1], in_=st[:m, :])
                nc.sync.dma_start(out=eler1[nt * P:nt * P + m, 0:2 * H1],
                                  in_=st[:m, F1:F1 + 2 * H1])

            if parts == "d1":
                zf = sp.tile([P, F2], F32, tag="zf")
                nc.vector.memset(zf[:], 0.0)
                for b in range(NB):
                    m = min(P, SHARD - b * P)
                    nc.sync.dma_start(out=outd[b * P:b * P + m, :],
                                      in_=zf[:m, :])

            # ---- phase E1 (+ fused dense layer 2) per 128-dst-node block ----
            if nb_run < NB:
                zf = sp.tile([P, F2], F32, tag="zf")
                nc.vector.memset(zf[:], 0.0)
                for b in range(NB):
                    m = min(P, SHARD - b * P)
                    nc.sync.dma_start(out=outd[b * P:b * P + m, :],
                                      in_=zf[:m, :])
            for b in range(nb_run if parts != "d1" else 0):
                m = min(P, SHARD - b * P)

                g = gp.tile([P, t, A1], BF16, tag="g1")
                if parts != "e1e":
                    gather_cols(g, haug[0:NLO, :], ixsb, b, 0, tlo, A1)
                    gather_cols(g, haug[NLO:N, :], ixsb, b, tlo, thi, A1)
                else:
                    nc.vector.memset(g[:], 0.0)
                # er[dst] via clamped lo/hi full-row gathers + masked combine
                gl = gp.tile([P, t, A2], BF16, tag="gl")
                gh = gp.tile([P, t, A2], BF16, tag="gh")
                if parts != "e1g":
                    gather_cols(gl, eler1[0:NLO, :], dlsb, b, 0, t, A2)
                    gather_cols(gh, eler1[NLO:N, :], dhsb, b, 0, t, A2)
                else:
                    nc.vector.memset(gl[:], 0.0)
                    nc.vector.memset(gh[:], 0.0)
                mlb = (mlsb[:, b * t:(b + 1) * t]
                       .rearrange("p (f o) -> p f o", o=1)
                       .to_broadcast([P, t, H1]))
                mhb = (mhsb[:, b * t:(b + 1) * t]
                       .rearrange("p (f o) -> p f o", o=1)
                       .to_broadcast([P, t, H1]))
                ere = mp.tile([P, t, H1], BF16, tag="ere")
                erh = mp.tile([P, t, H1], BF16, tag="erh")
                nc.vector.tensor_tensor(out=ere[:], in0=gl[:, :, H1:2 * H1],
                                        in1=mlb, op=OP.mult)
                nc.vector.tensor_tensor(out=erh[:], in0=gh[:, :, H1:2 * H1],
                                        in1=mhb, op=OP.mult)
                nc.vector.tensor_tensor(out=ere[:], in0=ere[:], in1=erh[:],
                                        op=OP.add)

                # e = el[src] + er[dst]; leaky_relu; w = exp(e) -> el slot
                nc.vector.tensor_tensor(out=ere[:], in0=g[:, :, F1:F1 + H1],
                                        in1=ere[:], op=OP.add)
                tmp = mp.tile([P, t, H1], BF16, tag="tmp")
                nc.vector.tensor_scalar_mul(out=tmp[:], in0=ere[:], scalar1=SLOPE)
                nc.vector.tensor_tensor(out=ere[:], in0=ere[:], in1=tmp[:],
                                        op=OP.max)
                nc.scalar.activation(out=g[:, :, F1:F1 + H1], in_=ere[:],
                                     func=AF.Exp)
                # h[src] *= w (per head)
                g4 = g[:, :, 0:F1].rearrange("p t (h d) -> p t h d", h=H1)
                wb = (g[:, :, F1:F1 + H1]
                      .rearrange("p t (h o) -> p t h o", o=1)
                      .to_broadcast([P, t, H1, HID]))
                nc.vector.tensor_tensor(out=g4, in0=g4, in1=wb, op=OP.mult)

                # selection matrices for all t columns in one op
                sel = selp.tile([P, t, P], BF16, tag="sel")
                in0 = (sgsb[:, b * t:(b + 1) * t]
                       .rearrange("p (f o) -> p f o", o=1).to_broadcast([P, t, P]))
                in1 = iosb.rearrange("p (o f) -> p o f", o=1).to_broadcast([P, t, P])
                nc.vector.tensor_tensor(out=sel[:], in0=in0, in1=in1,
                                        op=OP.is_equal)

                # segment-sum via matmuls accumulating in PSUM
                pb = ppa.tile([P, F1 + H1], F32, space="PSUM", tag="acc")
                for tt in range(t):
                    nc.tensor.matmul(out=pb[:], lhsT=sel[:, tt, :],
                                     rhs=g[:, tt, 0:F1 + H1],
                                     start=(tt == 0), stop=(tt == t - 1))

                # normalize: out = agg / denom (+eps keeps empty nodes at 0)
                den = mp.tile([P, H1], F32, tag="den")
                nc.vector.tensor_scalar_add(out=den[:m], in0=pb[:m, F1:F1 + H1],
                                            scalar1=1e-30)
                nc.vector.reciprocal(out=den[:m], in_=den[:m])
                h1t = hp.tile([P, F1], F32, tag="h1t")
                nc.vector.tensor_tensor(
                    out=h1t[:m].rearrange("p (h d) -> p h d", h=H1),
                    in0=pb[:m, 0:F1].rearrange("p (h d) -> p h d", h=H1),
                    in1=den[:m].rearrange("p (h o) -> p h o", o=1)
                        .to_broadcast([m, H1, HID]),
                    op=OP.mult)
                if not skip_b1:
                    nc.vector.tensor_tensor(out=h1t[:m], in0=h1t[:m],
                                            in1=b1sb[:m], op=OP.add)
                # elu+1 (the -1 is folded into bc2): max(h,0) + exp(min(h,0))
                te = hp.tile([P, F1], F32, tag="te")
                nc.vector.tensor_scalar_min(out=te[:m], in0=h1t[:m], scalar1=0.0)
                nc.scalar.activation(out=te[:m], in_=te[:m], func=AF.Exp)
                nc.vector.tensor_scalar_max(out=h1t[:m], in0=h1t[:m], scalar1=0.0)
                nc.vector.tensor_tensor(out=h1t[:m], in0=h1t[:m], in1=te[:m],
                                        op=OP.add)

                # transpose h1 and project: h2aug = h1 @ w2aug + bc2
                h1T = hp.tile([P, ki2, P], BF16, tag="h1T")
                for k in range(ki2):
                    pt = ppb.tile([P, P], F32, space="PSUM", tag="pt")
                    nc.tensor.transpose(out=pt[:, :m],
                                        in_=h1t[:m, k * P:(k + 1) * P],
                                        identity=idn[:m, :m])
                    nc.scalar.copy(out=h1T[:, k, :m], in_=pt[:, :m])
                p2 = ppc.tile([P, C2], F32, space="PSUM", tag="p2")
                for k in range(ki2):
                    nc.tensor.matmul(out=p2[:m, :], lhsT=h1T[:, k, :m],
                                     rhs=w2sb[:, k, :],
                                     start=(k == 0), stop=(k == ki2 - 1))
                st2 = sp.tile([P, C2], BF16, tag="st2")
                nc.vector.tensor_tensor(out=st2[:m, :], in0=p2[:m, :],
                                        in1=bc2sb[:m, :], op=OP.add)
                nc.sync.dma_start(out=h2loc[b * P:b * P + m, 0:C2], in_=st2[:m, :])
                if parts in ("e1", "e1g", "e1e"):
                    of = sp.tile([P, F2], F32, tag="of")
                    nc.scalar.copy(out=of[:m, :], in_=st2[:m, 0:F2])
                    nc.sync.dma_start(out=outd[b * P:b * P + m, :],
                                      in_=of[:m, :])

            # ---- phase C2: all-gather layer-2 table ----
            run_e2 = parts == "full"
            if run_e2:
                if nocoll:
                    nc.sync.dma_start(out=h2full[0:SHARD, :], in_=h2loc)
                else:
                    nc.gpsimd.collective_compute(
                        "AllGather", OP.bypass,
                        replica_groups=[list(range(NCORES))],
                        ins=[h2loc], outs=[h2full])

            # ---- phase E2: layer-2 edge phase -> final output ----
            for b in range(nb_run if run_e2 else 0):
                m = min(P, SHARD - b * P)

                g2 = g2p.tile([P, t, A2], BF16, tag="g2")
                gather_cols(g2, h2full[0:NLO, :], ixsb, b, 0, tlo, A2)
                gather_cols(g2, h2full[NLO:N, :], ixsb, b, tlo, thi, A2)
                gl2 = g2p.tile([P, t, A2], BF16, tag="gl2")
                gather_cols(gl2, h2full[0:NLO, :], dlsb, b, 0, t, A2)
                gh2 = g2p.tile([P, t, A2], BF16, tag="gh2")
                gather_cols(gh2, h2full[NLO:N, :], dhsb, b, 0, t, A2)
                mlb = (mlsb[:, b * t:(b + 1) * t]
                       .rearrange("p (f o) -> p f o", o=1)
                       .to_broadcast([P, t, H2]))
                mhb = (mhsb[:, b * t:(b + 1) * t]
                       .rearrange("p (f o) -> p f o", o=1)
                       .to_broadcast([P, t, H2]))
                er2 = mp.tile([P, t, H2], BF16, tag="er2")
                erh2 = mp.tile([P, t, H2], BF16, tag="erh2")
                nc.vector.tensor_tensor(out=er2[:],
                                        in0=gl2[:, :, F2 + H2:F2 + 2 * H2],
                                        in1=mlb, op=OP.mult)
                nc.vector.tensor_tensor(out=erh2[:],
                                        in0=gh2[:, :, F2 + H2:F2 + 2 * H2],
                                        in1=mhb, op=OP.mult)
                nc.vector.tensor_tensor(out=er2[:], in0=er2[:], in1=erh2[:],
                                        op=OP.add)

                nc.vector.tensor_tensor(out=er2[:], in0=g2[:, :, F2:F2 + H2],
                                        in1=er2[:], op=OP.add)
                tmp2 = mp.tile([P, t, H2], BF16, tag="tmp2")
                nc.vector.tensor_scalar_mul(out=tmp2[:], in0=er2[:], scalar1=SLOPE)
                nc.vector.tensor_tensor(out=er2[:], in0=er2[:], in1=tmp2[:],
                                        op=OP.max)
                nc.scalar.activation(out=er2[:], in_=er2[:], func=AF.Exp)
                # h2 *= w; denominator column <- w
                g2h = g2[:, :, 0:F2]
                w2b = er2[:].to_broadcast([P, t, F2])  # H2 == 1
                nc.vector.tensor_tensor(out=g2h, in0=g2h, in1=w2b, op=OP.mult)
                nc.scalar.copy(out=g2[:, :, F2:F2 + H2], in_=er2[:])

                sel = selp.tile([P, t, P], BF16, tag="sel")
                in0 = (sgsb[:, b * t:(b + 1) * t]
                       .rearrange("p (f o) -> p f o", o=1).to_broadcast([P, t, P]))
                in1 = iosb.rearrange("p (o f) -> p o f", o=1).to_broadcast([P, t, P])
                nc.vector.tensor_tensor(out=sel[:], in0=in0, in1=in1,
                                        op=OP.is_equal)

                pb2 = ppa.tile([P, F2 + H2], F32, space="PSUM", tag="acc")
                for tt in range(t):
                    nc.tensor.matmul(out=pb2[:], lhsT=sel[:, tt, :],
                                     rhs=g2[:, tt, 0:F2 + H2],
                                     start=(tt == 0), stop=(tt == t - 1))

                den2 = mp.tile([P, H2], F32, tag="den2")
                nc.vector.tensor_scalar_add(out=den2[:m], in0=pb2[:m, F2:F2 + H2],
                                            scalar1=1e-30)
                nc.vector.reciprocal(out=den2[:m], in_=den2[:m])
                of = sp.tile([P, F2], F32, tag="of")
                nc.vector.tensor_tensor(
                    out=of[:m].rearrange("p (h d) -> p h d", h=H2),
                    in0=pb2[:m, 0:F2].rearrange("p (h d) -> p h d", h=H2),
                    in1=den2[:m].rearrange("p (h o) -> p h o", o=1)
                        .to_broadcast([m, H2, F2]),
                    op=OP.mult)
                if not skip_b2:
                    nc.vector.tensor_tensor(out=of[:m], in0=of[:m], in1=b2sb[:m],
                                            op=OP.add)
                nc.sync.dma_start(out=outd[b * P:b * P + m, :], in_=of[:m, :])

    nc.compile()
    return nc


def make_inputs(features, src, dst, W1, al1, ar1, b1, W2, al2, ar2, b2):
    """Host-side preprocessing: per-core input dicts.
    Returns (in_maps, tlo, thi, skip_b1, skip_b2)."""
    features = np.asarray(features, dtype=np.float32)
    W1 = np.asarray(W1, dtype=np.float32)
    W2 = np.asarray(W2, dtype=np.float32)
    al1 = np.asarray(al1, dtype=np.float32).reshape(H1, HID)
    ar1 = np.asarray(ar1, dtype=np.float32).reshape(H1, HID)
    al2 = np.asarray(al2, dtype=np.float32).reshape(H2, OUT)
    ar2 = np.asarray(ar2, dtype=np.float32).reshape(H2, OUT)
    b1 = np.asarray(b1, dtype=np.float32).reshape(-1)
    b2 = np.asarray(b2, dtype=np.float32).reshape(-1)

    def blockdiag(a, heads, d):
        m = np.zeros((heads * d, heads), dtype=np.float32)
        for h in range(heads):
            m[h * d:(h + 1) * d, h] = a[h]
        return m

    w1aug = np.concatenate(
        [W1, W1 @ blockdiag(al1, H1, HID), W1 @ blockdiag(ar1, H1, HID)],
        axis=1)  # [IN, C1]
    w2aug = np.concatenate(
        [W2, W2 @ blockdiag(al2, H2, OUT), W2 @ blockdiag(ar2, H2, OUT)],
        axis=1)  # [F1, C2]

    # elu's -1 shift folded through w2aug: (X-1)@W = X@W - colsum(W)
    bc2 = np.tile(-w2aug.sum(axis=0, keepdims=True), (P, 1)).astype(np.float32)
    b1e = np.tile(b1[None, :], (P, 1)).astype(np.float32)
    b2e = np.tile(b2[None, :], (P, 1)).astype(np.float32)
    iota = np.tile(np.arange(P, dtype=np.float32)[None, :], (P, 1))

    skip_b1 = not np.any(b1)
    skip_b2 = not np.any(b2)

    xTb = np.ascontiguousarray(features.T).astype(ml_dtypes.bfloat16)
    w1ab = np.ascontiguousarray(w1aug).astype(ml_dtypes.bfloat16)
    w2ab = np.ascontiguousarray(w2aug).astype(ml_dtypes.bfloat16)
    iotab = iota.astype(ml_dtypes.bfloat16)

    (idx16s, dlo16s, dhi16s, segids, mlos, mhis,
     tlo, thi) = preprocess_edges(src, dst)

    in_maps = []
    for c in range(NCORES):
        in_maps.append({
            "xTb": xTb,
            "w1a": w1ab,
            "w2a": w2ab,
            "b1e": b1e,
            "bc2": bc2,
            "b2e": b2e,
            "iota": iotab,
            "idxs": idx16s[c],
            "dlo": dlo16s[c],
            "dhi": dhi16s[c],
            "segi": segids[c].astype(ml_dtypes.bfloat16),
            "mlo": mlos[c].astype(ml_dtypes.bfloat16),
            "mhi": mhis[c].astype(ml_dtypes.bfloat16),
        })
    return in_maps, tlo, thi, skip_b1, skip_b2


def _run(features, src, dst, W1, al1, ar1, b1, W2, al2, ar2, b2, **spmd_kwargs):
    in_maps, tlo, thi, skip_b1, skip_b2 = make_inputs(
        features, src, dst, W1, al1, ar1, b1, W2, al2, ar2, b2)
    nc = build_program(tlo, thi, skip_b1, skip_b2)
    res = run_bass_kernel_spmd(nc, in_maps, core_ids=list(range(NCORES)),
                               **spmd_kwargs)
    out = np.concatenate([res.results[c]["out"] for c in range(NCORES)], axis=0)
    return out.astype(np.float32), res


def kernel(features, src, dst, W1, al1, ar1, b1, W2, al2, ar2, b2):
    out, _ = _run(features, src, dst, W1, al1, ar1, b1, W2, al2, ar2, b2)
    return out


def run_timed(features, src, dst, W1, al1, ar1, b1, W2, al2, ar2, b2,
              iters=5):
    """Run like kernel(), but keep inputs device-resident and time repeated
    executions of the compiled NEFF. Returns (out, best_wall_ns)."""
    import time as _time

    import jax
    from jax.sharding import Mesh, PartitionSpec
    from jax.experimental.shard_map import shard_map
    from concourse.bass2jax import (_bass_exec_p, install_neuronx_cc_hook,
                                    partition_id_tensor)

    in_maps, tlo, thi, skip_b1, skip_b2 = make_inputs(
        features, src, dst, W1, al1, ar1, b1, W2, al2, ar2, b2)
    nc = build_program(tlo, thi, skip_b1, skip_b2)

    install_neuronx_cc_hook()
    part_name = (nc.partition_id_tensor.name if nc.partition_id_tensor
                 else None)
    in_names, out_names, out_avals, zero_outs = [], [], [], []
    for alloc in nc.m.functions[0].allocations:
        if not isinstance(alloc, mybir.MemoryLocationSet):
            continue
        name = alloc.memorylocations[0].name
        if alloc.kind == "ExternalInput":
            if name != part_name:
                in_names.append(name)
        elif alloc.kind == "ExternalOutput":
            out_names.append(name)
            shp = tuple(alloc.tensor_shape)
            dt = mybir.dt.np(alloc.dtype)
            out_avals.append(jax.core.ShapedArray(shp, dt))
            zero_outs.append(np.zeros(shp, dt))
    n_params = len(in_names)
    all_names = in_names + out_names
    if part_name is not None:
        all_names = all_names + [part_name]

    def _body(*args):
        operands = list(args)
        if part_name is not None:
            operands.append(partition_id_tensor())
        return tuple(_bass_exec_p.bind(
            *operands, out_avals=tuple(out_avals), in_names=tuple(all_names),
            out_names=tuple(out_names), lowering_input_output_aliases=(),
            sim_require_finite=True, sim_require_nnan=True, nc=nc))

    devices = jax.devices()[:NCORES]
    mesh = Mesh(np.asarray(devices), ("core",))
    specs = (PartitionSpec("core"),) * (n_params + len(out_names))
    out_specs = (PartitionSpec("core"),) * len(out_names)
    fn = jax.jit(shard_map(_body, mesh=mesh, in_specs=specs,
                           out_specs=out_specs, check_rep=False),
                 keep_unused=True)

    concat_in = [np.concatenate([in_maps[c][nm] for c in range(NCORES)], axis=0)
                 for nm in in_names]
    concat_zero = [np.concatenate([z] * NCORES, axis=0) for z in zero_outs]
    args = [jax.device_put(a) for a in concat_in + concat_zero]
    outs = fn(*args)  # compile + warm up
    jax.block_until_ready(outs)

    def timed_chain(k):
        best = None
        for _ in range(iters):
            t0 = _time.perf_counter_ns()
            for _ in range(k):
                outs = fn(*args)
            jax.block_until_ready(outs)
            dt = _time.perf_counter_ns() - t0
            best = dt if best is None else min(best, dt)
        return best

    t1 = timed_chain(1)
    t11 = timed_chain(11)
    slope = max((t11 - t1) // 10, 1)
    print(f"[timing] 1-call wall: {t1/1e6:.2f} ms; 11-call wall: "
          f"{t11/1e6:.2f} ms; marginal per-exec: {slope/1e6:.3f} ms",
          flush=True)
    outs = fn(*args)
    jax.block_until_ready(outs)
    out_full = np.asarray(outs[out_names.index("out")])
    return out_full.astype(np.float32), slope
